# revision 29
# baseline (speedup 1.0000x reference)
"""Trainium2 Bass kernel for the AttentionModel (encoder + LSTM + dot-attention
+ vocab projection), SPMD across 8 NeuronCores.

Sharding: encoder/LSTM/attention replicated on every core over the full batch
(B=32); the [32000, 2048] projection is sharded over the vocab dim (4000
rows/core, padded to 4096). No collectives.

v3 over v1:
- head restructured: whh/wih DMAs start immediately; h0 as soon as src means
  are done. Target embeddings are fetched with dma_gather(transpose=True),
  which lands them directly in the transposed xT layout [128, 4, tokens] —
  no PE transposes, no DRAM staging; 8-step chunks are re-gathered inside
  the loop on the otherwise-idle Pool engine (double-buffered ring).
- emit_h issues set0 (i,g) for all kc first, then set1 (f,o), so ACT starts
  at half-wave time.
- P6 split: pass1 (kc 8..15 = lstm_out half, vocab cols 0..2559) runs inside
  the LSTM loop as (m, vc) chunks of 8 matmuls placed between emit_x(t+1)
  and the tail-gated transposes, filling the PE wait gap; partials (+bias)
  spill to DRAM as bf16. Pacing is tuned empirically: 56 chunks in-loop,
  the rest right after the loop where they overlap the attention pipeline
  fill. pass2 (kc 0..7 = context half for vc 0..4 merging the spill, full
  16 kc for vc 5..7) runs after attention.
- projection weights streamed in phase-sized chunks that prefetch under the
  preceding phase (q1 under the head, C1 under P5, C23 under pass2-A).
"""

import os
import sys

sys.path.insert(0, "/opt/trn_rl_repo")

import numpy as np

import concourse.bass as bass
import concourse.tile as tile
from concourse import bacc, mybir
from concourse.bass import ts, ds
from concourse.bass_utils import run_bass_kernel_spmd
from concourse.masks import make_identity


B, S, T, E, H = 32, 100, 64, 512, 1024
SP = 128          # padded source length
VS = VT = 32000
NCORES = 8
VSH = VT // NCORES        # 4000 vocab rows per core
VSHP = 4096               # padded to 128 multiple
TOK = B * T               # 2048 tokens, t-major: j = t*32 + b
G4 = 4 * H                # 4096 gate width
F32 = mybir.dt.float32
BF16 = mybir.dt.bfloat16
I16 = mybir.dt.int16
I32 = mybir.dt.int32

NEG = -1e30
AF = mybir.ActivationFunctionType
ALU = mybir.AluOpType

# n-chunk -> (set, quadrant): set0 = {i(n0,n1), g(n4,n5)}, set1 = {f, o}
N2COL = {0: [0, 1, 4, 5], 1: [2, 3, 6, 7]}


def _wrap_idx(idx_flat: np.ndarray) -> np.ndarray:
    """Host: wrap flat indices into the [128, n/16] int16 layout dma_gather
    wants (index j at [j%16, j//16], replicated over the 8 groups of 16
    partitions)."""
    n = idx_flat.shape[0]
    assert n % 16 == 0
    w = idx_flat.astype(np.int16).reshape(n // 16, 16).T  # [16, n/16]
    return np.tile(w, (8, 1)).copy()                      # [128, n/16]


def build_nc():
    nc = bacc.Bacc("TRN2", target_bir_lowering=False, debug=False,
                   enable_asserts=False, num_devices=NCORES)

    # ---- parameters (all weights bf16, host-prepped) --------------------
    p_idx_src = nc.dram_tensor("idx_src", [128, B * SP // 16], I16, kind="ExternalInput")
    p_idx_tgt = nc.dram_tensor("idx_tgt", [128, TOK // 16], I16, kind="ExternalInput")
    p_emb_in = nc.dram_tensor("emb_in", [VS, E], BF16, kind="ExternalInput")
    p_emb_out = nc.dram_tensor("emb_out", [VT, E], BF16, kind="ExternalInput")
    p_pos = nc.dram_tensor("pos", [S, E], BF16, kind="ExternalInput")
    p_wh0T = nc.dram_tensor("wh0T", [E, H], BF16, kind="ExternalInput")
    p_beff = nc.dram_tensor("beff", [1, H], BF16, kind="ExternalInput")
    p_wihT = nc.dram_tensor("wihT", [128, 4, G4], BF16, kind="ExternalInput")
    p_whhT = nc.dram_tensor("whhT", [128, 8, G4], BF16, kind="ExternalInput")
    p_bsum = nc.dram_tensor("bsum", [1, G4], BF16, kind="ExternalInput")
    p_wp = nc.dram_tensor("wp", [128, 16, 8, 512], BF16, kind="ExternalInput")
    p_bpw = nc.dram_tensor("bpw", [128, VSHP], BF16, kind="ExternalInput")
    p_len = nc.dram_tensor("lens", [B, 1], F32, kind="ExternalInput")
    p_out = nc.dram_tensor("out", [TOK, VSHP], F32, kind="ExternalOutput")

    # ---- internal DRAM --------------------------------------------------
    d_encw = nc.dram_tensor("d_encw", [B, 128, E], BF16)   # word half of enc
    d_encT = nc.dram_tensor("d_encT", [B, 128, 4, S], BF16)
    d_negm = nc.dram_tensor("d_negm", [B, S], BF16)
    d_p1 = nc.dram_tensor("d_p1", [16, 5, 128, 512], BF16)  # pass1 spill

    _ts = bool(int(os.environ.get("SIMTRACE", "0")))
    CUT = int(os.environ.get("KCUT", "9"))
    with tile.TileContext(nc, trace_sim=_ts) as tc, \
         tc.tile_pool(name="consts", bufs=1) as consts, \
         tc.tile_pool(name="persist", bufs=1) as persist:

        ident_b = consts.tile([128, 128], BF16)
        make_identity(nc, ident_b)

        hsT = persist.tile([128, 8, T + 1, B], BF16)
        posT = consts.tile([128, 4, S], BF16)      # pos half of encT, shared
        pos_sb = consts.tile([128, E], BF16)       # pos rows, s on partitions
        c_fold = consts.tile([2 * B, 512], F32)
        ones1 = consts.tile([1, 32], BF16)
        nc.gpsimd.memset(ones1, 1.0)
        negm = consts.tile([B, S], BF16)           # -1e30 mask rows
        ones_col = consts.tile([1, T], BF16)
        nc.gpsimd.memset(ones_col, 1.0)
        bpw_lo = consts.tile([128, 5 * 512], BF16)  # proj bias, vc 0..4
        idx_tgt = consts.tile([128, TOK // 16], I16)

        with tc.tile_pool(name="p3w", bufs=1) as p3w:
            wih = p3w.tile([128, 4, G4], BF16)
            whh = p3w.tile([128, 8, G4], BF16)
            bias_bc = p3w.tile([1, G4], BF16)

            # critical-path DMAs first
            nc.sync.dma_start(whh[:], p_whhT[:])
            nc.sync.dma_start(wih[:], p_wihT[:])
            nc.sync.dma_start(bias_bc[:], p_bsum[:])
            nc.sync.dma_start(bpw_lo[:], p_bpw[:, 0:2560])

            # ================= head ======================================
            with tc.tile_pool(name="ph", bufs=2) as ph, \
                 tc.tile_pool(name="php", bufs=1, space="PSUM") as php:
                nc.sync.dma_start(idx_tgt[:], p_idx_tgt[:])

                # --- src gathers -> means + enc transposes (inline)
                pos0 = ph.tile([128, E], BF16, tag="pos0")
                nc.gpsimd.memset(pos0, 0.0)
                nc.sync.dma_start(pos0[0:S, :], p_pos[:])
                nc.vector.tensor_copy(pos_sb[:], pos0[:])
                idx_src = ph.tile([128, B * SP // 16], I16, tag="idxs")
                nc.sync.dma_start(idx_src[:], p_idx_src[:])
                ones_s = ph.tile([128, 1], BF16, tag="ones_s")
                nc.gpsimd.memset(ones_s, 0.0)
                nc.gpsimd.memset(ones_s[0:S], 1.0)

                NB_CH = 8
                with tc.tile_pool(name="pmean", bufs=1, space="PSUM") as pm:
                    ps_mean = pm.tile([128, 4, B], F32)
                    for half in range(B // NB_CH):
                        g_in = ph.tile([128, NB_CH, E], BF16, tag="gin")
                        nc.gpsimd.dma_gather(
                            g_in[:], p_emb_in[:, :],
                            idx_src[:, ts(half, NB_CH * SP // 16)],
                            NB_CH * SP, NB_CH * SP, E)
                        nc.sync.dma_start(
                            d_encw.ap()[half * NB_CH:(half + 1) * NB_CH]
                            .rearrange("b s e -> s b e")[:, :, :], g_in[:])
                        for bi in range(NB_CH):
                            b = half * NB_CH + bi
                            psx = php.tile([128, 4, 128], BF16, tag="xtp")
                            for kc in range(4):
                                nc.tensor.transpose(psx[:, kc, :],
                                                    g_in[:, bi, ts(kc, 128)],
                                                    ident_b)
                                nc.tensor.matmul(ps_mean[:, kc, b:b + 1],
                                                 g_in[:, bi, ts(kc, 128)],
                                                 ones_s[:], start=True,
                                                 stop=True)
                            e_st = ph.tile([128, 4, S], BF16, tag="est")
                            nc.vector.tensor_copy(e_st[:], psx[:, :, 0:S])
                            nc.sync.dma_start(d_encT.ap()[b], e_st[:])
                    meanTb = ph.tile([128, 4, B], BF16, tag="mtb")
                    nc.scalar.mul(meanTb[:], ps_mean[:], 1.0 / S)

                # --- h0 = mean_w @ W_h0w.T + b_eff
                beff = ph.tile([1, H], BF16, tag="beff")
                nc.sync.dma_start(beff[:], p_beff[:])
                h_b16 = ph.tile([2 * B, 512], BF16, tag="hb16")
                for n in range(2):
                    w0 = ph.tile([128, 4, 512], BF16, tag="w0")
                    nc.sync.dma_start(
                        w0[:], p_wh0T.ap()
                        .rearrange("(kc p) h -> p kc h", p=128)[:, :, ts(n, 512)])
                    ps = php.tile([B, 512], F32, tag="hid")
                    for kc in range(4):
                        nc.tensor.matmul(ps[:], meanTb[:, kc], w0[:, kc],
                                         start=(kc == 0), stop=False)
                    nc.tensor.matmul(ps[:], ones1[0:1, :],
                                     beff[:, ts(n, 512)],
                                     start=False, stop=True)
                    nc.vector.tensor_copy(c_fold[n * B:(n + 1) * B, :], ps[:])
                    nc.vector.tensor_copy(h_b16[n * B:(n + 1) * B, :], ps[:])
                pst0 = php.tile([128, 8, B], BF16, tag="htp0")
                for kc in range(8):
                    hf, q = kc // 4, kc % 4
                    nc.tensor.transpose(
                        pst0[:, kc, :], h_b16[32 * hf:32 * (hf + 1), ts(q, 128)],
                        ident_b[32 * hf:32 * (hf + 1), 32 * hf:32 * (hf + 1)])
                nc.vector.tensor_copy(hsT[:, :, 0, :], pst0[:])

                # --- negmask from lengths (cheap, off critical path)
                lens = ph.tile([B, 1], F32, tag="lens")
                nc.sync.dma_start(lens[:], p_len[:])
                iota_i = ph.tile([B, S], I32, tag="iota")
                nc.gpsimd.iota(iota_i[:], pattern=[[1, S]], base=0,
                               channel_multiplier=0)
                iota_f = ph.tile([B, S], F32, tag="iotaf")
                nc.vector.tensor_copy(iota_f[:], iota_i[:])
                sg = ph.tile([B, S], F32, tag="sg")
                nc.vector.tensor_scalar(sg[:], iota_f[:], lens[:], None,
                                        ALU.subtract)
                halfc = ph.tile([B, 1], F32, tag="halfc")
                nc.gpsimd.memset(halfc, 0.5)
                negc = ph.tile([B, 1], F32, tag="negc")
                nc.gpsimd.memset(negc, NEG / 2)
                sg2 = ph.tile([B, S], F32, tag="sg2")
                nc.scalar.activation(sg2[:], sg[:], AF.Sign, bias=halfc[:])
                nc.scalar.activation(negm[:], sg2[:], AF.Identity,
                                     bias=negc[:], scale=NEG / 2)
                nc.sync.dma_start(d_negm[:], negm[:])

                # posT transposes (shared pos half of encT)
                for kc in range(4):
                    pst = php.tile([128, 128], BF16, tag="ptp")
                    nc.tensor.transpose(pst[:], pos_sb[:, ts(kc, 128)], ident_b)
                    nc.vector.tensor_copy(posT[:, kc], pst[:, 0:S])

            # ================= LSTM loop =================================
            with tc.tile_pool(name="wq1", bufs=1) as wq1, \
                 tc.tile_pool(name="p4s", bufs=2) as p4s, \
                 tc.tile_pool(name="pxb", bufs=2) as pxb, \
                 tc.tile_pool(name="p4ps", bufs=2, space="PSUM") as p4ps, \
                 tc.tile_pool(name="p4pt", bufs=1, space="PSUM") as p4pt, \
                 tc.tile_pool(name="p1ps", bufs=3, space="PSUM") as p1ps:

                # pass1 weights: kc 8..15, vocab cols 0..2047
                wp_q1 = wq1.tile([128, 8, 5, 512], BF16)
                nc.sync.dma_start(wp_q1[:], p_wp[:, 8:16, 0:5, :])

                XC = 8  # steps per transposed-gather chunk
                xbufs = {}

                def fetch_x(ch):
                    xb = pxb.tile([128, 4, XC * 32], BF16, tag="xb",
                                  name=f"xb{ch}")
                    nc.gpsimd.dma_gather(xb[:], p_emb_out[:, :],
                                         idx_tgt[:, ts(ch, TOK // 128)],
                                         TOK // 8, TOK // 8, E,
                                         transpose=True)
                    xbufs[ch] = xb

                def emit_x(t, pss):
                    """x + bias waves for step t (h-independent)."""
                    xb = xbufs[t // XC]
                    toff = (t % XC) * 32
                    for i in range(5):
                        for s_ in range(2):
                            ps = pss[s_]
                            for j in range(4):
                                n = N2COL[s_][j]
                                if i < 4:
                                    lhsT = xb[:, i, toff:toff + 32]
                                    rhs = wih[:, i, ts(n, 512)]
                                else:
                                    lhsT = ones1[:]
                                    rhs = bias_bc[0:1, ts(n, 512)]
                                nc.tensor.matmul(
                                    ps[32 * j:32 * (j + 1), :], lhsT, rhs,
                                    start=(i == 0), stop=False,
                                    skip_group_check=True,
                                    tile_position=(0, 32 * j))

                def emit_h(t, pss, acts):
                    """h waves: all kc of set0 first, then set1; then gate
                    activations per set."""
                    for s_ in range(2):
                        ps = pss[s_]
                        for kc in range(8):
                            for j in range(4):
                                n = N2COL[s_][j]
                                nc.tensor.matmul(
                                    ps[32 * j:32 * (j + 1), :], hsT[:, kc, t],
                                    whh[:, kc, ts(n, 512)],
                                    start=False, stop=(kc == 7),
                                    skip_group_check=True,
                                    tile_position=(0, 32 * j))
                        alo = p4s.tile([64, 512], BF16, tag=f"alo{s_}")
                        nc.scalar.activation(alo[:], ps[0:64, :], AF.Sigmoid)
                        ahi = p4s.tile([64, 512], BF16, tag=f"ahi{s_}")
                        nc.scalar.activation(ahi[:], ps[64:128, :],
                                             AF.Tanh if s_ == 0 else AF.Sigmoid)
                        acts.extend([alo, ahi])

                def emit_tail(t, acts):
                    """c/h update."""
                    gi_, gg, gf, go = acts
                    t2 = p4s.tile([64, 512], BF16, tag="t2")
                    nc.vector.tensor_mul(t2[:], gi_[:], gg[:])
                    nc.vector.tensor_mul(c_fold[:], gf[:], c_fold[:])
                    nc.vector.tensor_add(c_fold[:], c_fold[:], t2[:])
                    thc = p4s.tile([64, 512], BF16, tag="thc")
                    nc.scalar.activation(thc[:], c_fold[:], AF.Tanh)
                    h_fold = p4s.tile([64, 512], BF16, tag="hf")
                    nc.vector.tensor_mul(h_fold[:], go[:], thc[:])
                    return h_fold

                def emit_transposes(t, h_fold):
                    for g in range(2):
                        pst = p4pt.tile([128, 4, B], BF16, tag="htp")
                        for qq in range(4):
                            kc = g * 4 + qq
                            hf, q = kc // 4, kc % 4
                            nc.tensor.transpose(
                                pst[:, qq, :],
                                h_fold[32 * hf:32 * (hf + 1), ts(q, 128)],
                                ident_b[32 * hf:32 * (hf + 1),
                                        32 * hf:32 * (hf + 1)])
                        nc.vector.tensor_copy(
                            hsT[:, g * 4:(g + 1) * 4, t + 1, :], pst[:])

                def emit_pass1(m, vc):
                    """one pass1 chunk: spill[m, vc] = lstm_out @ Wp + bias."""
                    ps = p1ps.tile([128, 512], F32, tag="p1",
                                   name=f"p1_{m}_{vc}")
                    for kc in range(8):
                        nc.tensor.matmul(ps[:],
                                         hsT[:, kc, 1 + m * 4:5 + m * 4, :],
                                         wp_q1[:, kc, vc, :],
                                         start=(kc == 0), stop=(kc == 7))
                    sp = p4s.tile([128, 512], BF16, tag="sp")
                    nc.vector.scalar_tensor_tensor(
                        sp[:], ps[:], 1.0, bpw_lo[:, ts(vc, 512)],
                        ALU.mult, ALU.add)
                    nc.sync.dma_start(d_p1.ap()[m, vc], sp[:])

                NP1 = 80  # pass1 chunks: 16 m x 5 vc
                k_em = [0]

                def pace_pass1(t):
                    target = min(NP1 - 24, ((t - 3) * (NP1 - 24) + 59) // 60)
                    mmax = (t - 4) // 4
                    while k_em[0] < target and k_em[0] // 5 <= mmax:
                        emit_pass1(k_em[0] // 5, k_em[0] % 5)
                        k_em[0] += 1

                fetch_x(0)
                fetch_x(1)
                ps_cur = [p4ps.tile([128, 512], F32, tag=f"g{s_}",
                                    name=f"gates{s_}")
                          for s_ in range(2)]
                emit_x(0, ps_cur)
                for t in range(T if CUT >= 2 else 0):
                    acts = []
                    emit_h(t, ps_cur, acts)
                    if t + 1 < T:
                        if (t + 1) % XC == 0 and (t + 1) // XC + 1 < T // XC:
                            fetch_x((t + 1) // XC + 1)

                        ps_nxt = [p4ps.tile([128, 512], F32, tag=f"g{s_}",
                                            name=f"gates{s_}_{t + 1}")
                                  for s_ in range(2)]
                        emit_x(t + 1, ps_nxt)
                    # PE filler between emit_x and the tail-gated transposes
                    if t >= 4:
                        pace_pass1(t)
                    h_fold = emit_tail(t, acts)
                    emit_transposes(t, h_fold)
                    if t + 1 < T:
                        ps_cur = ps_nxt
                while k_em[0] < NP1:  # pass1 leftovers
                    emit_pass1(k_em[0] // 5, k_em[0] % 5)
                    k_em[0] += 1

        # ================= P5 (attention) + P6 pass2 =====================
        with tc.tile_pool(name="p56", bufs=1) as p56:
            combT = p56.tile([128, 8, T, B], BF16)
            wp_c1 = p56.tile([128, 8, 5, 512], BF16)   # kc 0..7, vc 0..4
            nc.sync.dma_start(wp_c1[:], p_wp[:, 0:8, 0:5, :])
            bpw_hi = p56.tile([128, 3 * 512], BF16)
            nc.sync.dma_start(bpw_hi[:], p_bpw[:, 2560:4096])

            with tc.tile_pool(name="p5c", bufs=1) as p5c, \
                 tc.tile_pool(name="p5", bufs=4) as p5, \
                 tc.tile_pool(name="p5e", bufs=3, space="PSUM") as p5e, \
                 tc.tile_pool(name="p5cx", bufs=2, space="PSUM") as p5cx:
                negm0 = p5c.tile([1, B * S], BF16)
                nc.sync.dma_start(negm0[:],
                                  d_negm.ap().rearrange("b s -> (b s)")[None, :])

                for b in range(B if CUT >= 3 else 0):
                    encw_b = p5.tile([128, E], BF16, tag="encwb")
                    nc.sync.dma_start(encw_b[:], d_encw[b])
                    encT_b = p5.tile([128, 4, S], BF16, tag="encTb")
                    nc.sync.dma_start(encT_b[:], d_encT.ap()[b])

                    ps_e = p5e.tile([T, S], F32, tag="eng")
                    for kc in range(8):
                        rhs = encT_b[:, kc] if kc < 4 else posT[:, kc - 4]
                        nc.tensor.matmul(ps_e[:], hsT[:, kc, 0:T, b], rhs,
                                         start=(kc == 0), stop=False)
                    nc.tensor.matmul(ps_e[:], ones_col[:],
                                     negm0[:, b * S:(b + 1) * S],
                                     start=False, stop=True)
                    expE = p5.tile([T, S], F32, tag="expE")
                    esum = p5.tile([T, 1], F32, tag="esum")
                    nc.scalar.activation(expE[:], ps_e[:], AF.Exp,
                                         accum_out=esum[:])
                    esc = p5.tile([T, 1], F32, tag="esc")
                    nc.scalar.mul(esc[:], esum[:], float(S))
                    erec = p5.tile([T, 1], F32, tag="erec")
                    nc.vector.reciprocal(erec[:], esc[:])
                    align = p5.tile([T, S], BF16, tag="align")
                    nc.scalar.activation(align[:], expE[:], AF.Copy,
                                         scale=erec[:])
                    ps_at = p5e.tile([128, T], BF16, tag="alT")
                    nc.tensor.transpose(ps_at[0:S, :], align[:],
                                        ident_b[0:T, 0:T])
                    alT = p5.tile([S, T], BF16, tag="alTs")
                    nc.vector.tensor_copy(alT[:], ps_at[0:S, :])
                    ps_c = p5cx.tile([128, 8, T], F32, tag="ctx")
                    for mc in range(8):
                        lhsT = (encw_b[0:S, ts(mc, 128)] if mc < 4
                                else pos_sb[0:S, ts(mc - 4, 128)])
                        nc.tensor.matmul(ps_c[:, mc, :], lhsT, alT[:],
                                         start=True, stop=True)
                    nc.vector.tensor_copy(combT[:, :, :, b], ps_c[:])

            # ---- pass2: phase A (vc 0..3, kc 0..7 + spill merge) --------
            with tc.tile_pool(name="p6w", bufs=1) as p6w, \
                 tc.tile_pool(name="p6", bufs=3) as p6, \
                 tc.tile_pool(name="p6s", bufs=2) as p6s, \
                 tc.tile_pool(name="p6ps", bufs=2, space="PSUM") as p6ps:
                # first spill block before the big phase-B weight DMA so it
                # isn't queued behind 8MB
                sp0 = p6s.tile([128, 4, 512], BF16, tag="spin", name="spin0")
                for vc in range(4):
                    nc.sync.dma_start(sp0[:, vc], d_p1.ap()[0, vc])
                # prefetch phase-B weights under phase A
                wp_c23 = p6w.tile([128, 16, 3, 512], BF16)
                nc.sync.dma_start(wp_c23[:], p_wp[:, :, 5:8, :])

                # kc-outer so the 4 vc matmuls share one stationary load
                for m in range(16 if CUT >= 4 else 0):
                    if m == 0:
                        sp_in = sp0
                    else:
                        sp_in = p6s.tile([128, 4, 512], BF16, tag="spin",
                                         name=f"spin{m}")
                        for vc in range(4):
                            nc.sync.dma_start(sp_in[:, vc], d_p1.ap()[m, vc])
                    pss = [p6ps.tile([128, 512], F32, tag=f"o{v}",
                                     name=f"psA_{m}_{v}") for v in range(4)]
                    for kc in range(8):
                        for vc in range(4):
                            nc.tensor.matmul(pss[vc][:],
                                             combT[:, kc, ts(m, 4), :],
                                             wp_c1[:, kc, vc, :],
                                             start=(kc == 0), stop=(kc == 7))
                    for vc in range(4):
                        o_sb = p6.tile([128, 512], F32, tag="osb")
                        nc.vector.scalar_tensor_tensor(
                            o_sb[:], pss[vc][:], 1.0, sp_in[:, vc],
                            ALU.mult, ALU.add)
                        nc.sync.dma_start(p_out[ts(m, 128), ts(vc, 512)],
                                          o_sb[:])

                # ---- pass2: phase B (vc4 spill-merge + vc 5..7 full) ----
                for m in range(16 if CUT >= 4 else 0):
                    sp4 = p6s.tile([128, 512], BF16, tag="sp4",
                                   name=f"sp4_{m}")
                    nc.sync.dma_start(sp4[:], d_p1.ap()[m, 4])
                    ps4 = p6ps.tile([128, 512], F32, tag="o0",
                                    name=f"psB4_{m}")
                    pss = [p6ps.tile([128, 512], F32, tag=f"o{v + 1}",
                                     name=f"psB_{m}_{v}") for v in range(3)]
                    for kc in range(16):
                        if kc < 8:
                            lhsT = combT[:, kc, ts(m, 4), :]
                        else:
                            lhsT = hsT[:, kc - 8, 1 + m * 4:1 + (m + 1) * 4, :]
                        if kc < 8:
                            nc.tensor.matmul(ps4[:], lhsT,
                                             wp_c1[:, kc, 4, :],
                                             start=(kc == 0), stop=(kc == 7))
                        for v in range(3):
                            nc.tensor.matmul(pss[v][:], lhsT,
                                             wp_c23[:, kc, v, :],
                                             start=(kc == 0), stop=(kc == 15))
                    o4 = p6.tile([128, 512], F32, tag="osb")
                    nc.vector.scalar_tensor_tensor(
                        o4[:], ps4[:], 1.0, sp4[:], ALU.mult, ALU.add)
                    nc.sync.dma_start(p_out[ts(m, 128), ts(4, 512)], o4[:])
                    for v in range(3):
                        o_sb = p6.tile([128, 512], F32, tag="osb")
                        nc.vector.scalar_tensor_tensor(
                            o_sb[:], pss[v][:], 1.0, bpw_hi[:, ts(v, 512)],
                            ALU.mult, ALU.add)
                        nc.sync.dma_start(p_out[ts(m, 128), ts(5 + v, 512)],
                                          o_sb[:])

    nc.finalize()
    return nc


_CACHED = {}
LAST_EXEC_NS = None


def prep_in_maps(inputs):
    import ml_dtypes
    bf16 = ml_dtypes.bfloat16

    src = np.asarray(inputs["source_sentences"]).astype(np.int64)
    lens = np.asarray(inputs["source_lengths"]).astype(np.float32).reshape(B, 1)
    tgt = np.asarray(inputs["target_sentences"]).astype(np.int64)
    emb_in = np.asarray(inputs["emb_in"], np.float32).astype(bf16)
    emb_out = np.asarray(inputs["emb_out"], np.float32).astype(bf16)
    pos_emb = np.asarray(inputs["pos_emb"], np.float32)[:S]
    w_h0 = np.asarray(inputs["W_h0"], np.float32)         # [H, 2E]
    b_h0 = np.asarray(inputs["b_h0"], np.float32)
    # fold the (batch-independent) positional mean through W_h0 into the bias
    pos_mean = pos_emb.mean(axis=0)                       # [E]
    b_eff = b_h0 + w_h0[:, E:] @ pos_mean                 # [H]
    wihT = np.asarray(inputs["W_ih"], np.float32).T       # [E, 4H]
    whhT = np.asarray(inputs["W_hh"], np.float32).T       # [H, 4H]
    bsum = (np.asarray(inputs["b_ih"], np.float32)
            + np.asarray(inputs["b_hh"], np.float32)).reshape(1, G4)
    wproj = np.asarray(inputs["W_proj"], np.float32)
    bproj = np.asarray(inputs["b_proj"], np.float32)

    src_pad = np.zeros((B, SP), np.int64)
    src_pad[:, :S] = src
    idx_src = _wrap_idx(src_pad.reshape(-1))
    idx_tgt = _wrap_idx(tgt.T.reshape(-1))                # t-major: j = t*32+b

    common = dict(
        idx_src=idx_src, idx_tgt=idx_tgt,
        emb_in=np.ascontiguousarray(emb_in),
        emb_out=np.ascontiguousarray(emb_out),
        pos=np.ascontiguousarray(pos_emb.astype(bf16)),
        wh0T=np.ascontiguousarray(w_h0[:, :E].T.astype(bf16)),
        beff=np.ascontiguousarray(b_eff.reshape(1, H).astype(bf16)),
        wihT=np.ascontiguousarray(
            wihT.reshape(4, 128, G4).transpose(1, 0, 2).astype(bf16)),
        whhT=np.ascontiguousarray(
            whhT.reshape(8, 128, G4).transpose(1, 0, 2).astype(bf16)),
        bsum=np.ascontiguousarray(bsum.astype(bf16)),
        lens=lens,
    )
    in_maps = []
    for c in range(NCORES):
        wp = wproj[c * VSH:(c + 1) * VSH]                 # [4000, 2048]
        wpT = np.zeros((2 * E + H, VSHP), np.float32)
        wpT[:, :VSH] = wp.T
        # pack [kc*128+p, vc8*512+v] -> [p, kc, vc8, v]
        wp_pk = np.ascontiguousarray(
            wpT.reshape(16, 128, 8, 512).transpose(1, 0, 2, 3).astype(bf16))
        bp = np.zeros((VSHP,), np.float32)
        bp[:VSH] = bproj[c * VSH:(c + 1) * VSH]
        bpw = np.ascontiguousarray(
            np.tile(bp.astype(bf16)[None, :], (128, 1)))
        in_maps.append(dict(common, wp=wp_pk, bpw=bpw))

    return in_maps


def kernel(**inputs) -> np.ndarray:
    in_maps = prep_in_maps(inputs)
    if "nc" not in _CACHED:
        _CACHED["nc"] = build_nc()
    nc = _CACHED["nc"]
    trace = bool(int(os.environ.get("KTRACE", "0")))
    tmpdir = os.environ.get("KTRACE_DIR") or None
    res = run_bass_kernel_spmd(nc, in_maps, list(range(NCORES)),
                               trace=trace, tmpdir=tmpdir)
    global LAST_EXEC_NS
    LAST_EXEC_NS = res.exec_time_ns
    outs = []
    for c in range(NCORES):
        o = res.results[c]["out"]                  # [TOK, VSHP] tok=t*32+b
        o = o.reshape(T, B, VSHP)[:, :, :VSH].transpose(1, 0, 2)
        outs.append(o)                             # [B, T, 4000]
    return np.concatenate(outs, axis=2)


if __name__ == "__main__":
    build_nc()
    print("build ok")


# revision 30
# speedup vs baseline: 1.2707x; 1.2707x over previous
"""Trainium2 Bass kernel for the AttentionModel (encoder + LSTM + dot-attention
+ vocab projection), SPMD across 8 NeuronCores.

Sharding: encoder/LSTM/attention replicated on every core over the full batch
(B=32); the [32000, 2048] projection is sharded over the vocab dim (4000
rows/core, padded to 4096). No collectives.

v3 over v1:
- head restructured: whh/wih DMAs start immediately; h0 as soon as src means
  are done. Target embeddings are fetched with dma_gather(transpose=True),
  which lands them directly in the transposed xT layout [128, 4, tokens] —
  no PE transposes, no DRAM staging; 8-step chunks are re-gathered inside
  the loop on the otherwise-idle Pool engine (double-buffered ring).
- emit_h issues set0 (i,g) for all kc first, then set1 (f,o), so ACT starts
  at half-wave time.
- P6 split: pass1 (kc 8..15 = lstm_out half, vocab cols 0..2559) runs inside
  the LSTM loop as (m, vc) chunks of 8 matmuls placed between emit_x(t+1)
  and the tail-gated transposes, filling the PE wait gap; partials (+bias)
  spill to DRAM as bf16. Pacing is tuned empirically: 56 chunks in-loop,
  the rest right after the loop where they overlap the attention pipeline
  fill. pass2 (kc 0..7 = context half for vc 0..4 merging the spill, full
  16 kc for vc 5..7) runs after attention.
- projection weights streamed in phase-sized chunks that prefetch under the
  preceding phase (q1 under the head, C1 under P5, C23 under pass2-A).
"""

import os
import sys

sys.path.insert(0, "/opt/trn_rl_repo")

import numpy as np

import concourse.bass as bass
import concourse.tile as tile
from concourse import bacc, mybir
from concourse.bass import ts, ds
from concourse.bass_utils import run_bass_kernel_spmd
from concourse.masks import make_identity


B, S, T, E, H = 32, 100, 64, 512, 1024
SP = 128          # padded source length
VS = VT = 32000
NCORES = 8
VSH = VT // NCORES        # 4000 vocab rows per core
VSHP = 4096               # padded to 128 multiple
TOK = B * T               # 2048 tokens, t-major: j = t*32 + b
G4 = 4 * H                # 4096 gate width
F32 = mybir.dt.float32
BF16 = mybir.dt.bfloat16
I16 = mybir.dt.int16
I32 = mybir.dt.int32

NEG = -1e30
AF = mybir.ActivationFunctionType
ALU = mybir.AluOpType

# n-chunk -> (set, quadrant): set0 = {i(n0,n1), g(n4,n5)}, set1 = {f, o}
N2COL = {0: [0, 1, 4, 5], 1: [2, 3, 6, 7]}


def _wrap_idx(idx_flat: np.ndarray) -> np.ndarray:
    """Host: wrap flat indices into the [128, n/16] int16 layout dma_gather
    wants (index j at [j%16, j//16], replicated over the 8 groups of 16
    partitions)."""
    n = idx_flat.shape[0]
    assert n % 16 == 0
    w = idx_flat.astype(np.int16).reshape(n // 16, 16).T  # [16, n/16]
    return np.tile(w, (8, 1)).copy()                      # [128, n/16]


def build_nc():
    nc = bacc.Bacc("TRN2", target_bir_lowering=False, debug=False,
                   enable_asserts=False, num_devices=NCORES)

    # ---- parameters (all weights bf16, host-prepped) --------------------
    p_idx_src = nc.dram_tensor("idx_src", [128, B * SP // 16], I16, kind="ExternalInput")
    p_idx_tgt = nc.dram_tensor("idx_tgt", [128, TOK // 16], I16, kind="ExternalInput")
    p_emb_in = nc.dram_tensor("emb_in", [VS, E], BF16, kind="ExternalInput")
    p_emb_out = nc.dram_tensor("emb_out", [VT, E], BF16, kind="ExternalInput")
    p_pos = nc.dram_tensor("pos", [S, E], BF16, kind="ExternalInput")
    p_wh0T = nc.dram_tensor("wh0T", [E, H], BF16, kind="ExternalInput")
    p_beff = nc.dram_tensor("beff", [1, H], BF16, kind="ExternalInput")
    p_wihT = nc.dram_tensor("wihT", [128, 4, G4], BF16, kind="ExternalInput")
    p_whhT = nc.dram_tensor("whhT", [128, 8, G4], BF16, kind="ExternalInput")
    p_bsum = nc.dram_tensor("bsum", [1, G4], BF16, kind="ExternalInput")
    p_wp = nc.dram_tensor("wp", [128, 16, 8, 512], BF16, kind="ExternalInput")
    p_bpw = nc.dram_tensor("bpw", [128, VSHP], BF16, kind="ExternalInput")
    p_len = nc.dram_tensor("lens", [B, 1], F32, kind="ExternalInput")
    p_out = nc.dram_tensor("out", [TOK, VSHP], F32, kind="ExternalOutput")

    # ---- internal DRAM --------------------------------------------------
    d_encw = nc.dram_tensor("d_encw", [B, 128, E], BF16)   # word half of enc
    d_encT = nc.dram_tensor("d_encT", [B, 128, 4, S], BF16)
    d_negm = nc.dram_tensor("d_negm", [B, S], BF16)
    d_p1 = nc.dram_tensor("d_p1", [16, 5, 128, 512], BF16)  # pass1 spill

    _ts = bool(int(os.environ.get("SIMTRACE", "0")))
    CUT = int(os.environ.get("KCUT", "9"))
    with tile.TileContext(nc, trace_sim=_ts) as tc, \
         tc.tile_pool(name="consts", bufs=1) as consts, \
         tc.tile_pool(name="persist", bufs=1) as persist:

        ident_b = consts.tile([128, 128], BF16)
        make_identity(nc, ident_b)

        hsT = persist.tile([128, 8, T + 1, B], BF16)
        posT = consts.tile([128, 4, S], BF16)      # pos half of encT, shared
        pos_sb = consts.tile([128, E], BF16)       # pos rows, s on partitions
        c_fold = consts.tile([2 * B, 512], F32)
        ones1 = consts.tile([1, 32], BF16)
        nc.gpsimd.memset(ones1, 1.0)
        negm = consts.tile([B, S], BF16)           # -1e30 mask rows
        ones_col = consts.tile([1, T], BF16)
        nc.gpsimd.memset(ones_col, 1.0)
        bpw_lo = consts.tile([128, 5 * 512], BF16)  # proj bias, vc 0..4
        idx_tgt = consts.tile([128, TOK // 16], I16)

        with tc.tile_pool(name="p3w", bufs=1) as p3w:
            wih = p3w.tile([128, 4, G4], BF16)
            whh = p3w.tile([128, 8, G4], BF16)
            bias_bc = p3w.tile([1, G4], BF16)

            # critical-path DMAs first
            nc.sync.dma_start(whh[:], p_whhT[:])
            nc.sync.dma_start(wih[:], p_wihT[:])
            nc.sync.dma_start(bias_bc[:], p_bsum[:])
            nc.sync.dma_start(bpw_lo[:], p_bpw[:, 0:2560])

            # ================= head ======================================
            with tc.tile_pool(name="ph", bufs=2) as ph, \
                 tc.tile_pool(name="php", bufs=1, space="PSUM") as php:
                nc.sync.dma_start(idx_tgt[:], p_idx_tgt[:])

                # --- src gathers -> means + enc transposes (inline)
                pos0 = ph.tile([128, E], BF16, tag="pos0")
                nc.gpsimd.memset(pos0, 0.0)
                nc.sync.dma_start(pos0[0:S, :], p_pos[:])
                nc.vector.tensor_copy(pos_sb[:], pos0[:])
                idx_src = ph.tile([128, B * SP // 16], I16, tag="idxs")
                nc.sync.dma_start(idx_src[:], p_idx_src[:])
                ones_s = ph.tile([128, 1], BF16, tag="ones_s")
                nc.gpsimd.memset(ones_s, 0.0)
                nc.gpsimd.memset(ones_s[0:S], 1.0)

                NB_CH = 8
                with tc.tile_pool(name="pmean", bufs=1, space="PSUM") as pm:
                    ps_mean = pm.tile([128, 4, B], F32)
                    for half in range(B // NB_CH):
                        g_in = ph.tile([128, NB_CH, E], BF16, tag="gin")
                        nc.gpsimd.dma_gather(
                            g_in[:], p_emb_in[:, :],
                            idx_src[:, ts(half, NB_CH * SP // 16)],
                            NB_CH * SP, NB_CH * SP, E)
                        nc.sync.dma_start(
                            d_encw.ap()[half * NB_CH:(half + 1) * NB_CH]
                            .rearrange("b s e -> s b e")[:, :, :], g_in[:])
                        for bi in range(NB_CH):
                            b = half * NB_CH + bi
                            psx = php.tile([128, 4, 128], BF16, tag="xtp")
                            for kc in range(4):
                                nc.tensor.transpose(psx[:, kc, :],
                                                    g_in[:, bi, ts(kc, 128)],
                                                    ident_b)
                                nc.tensor.matmul(ps_mean[:, kc, b:b + 1],
                                                 g_in[:, bi, ts(kc, 128)],
                                                 ones_s[:], start=True,
                                                 stop=True)
                            e_st = ph.tile([128, 4, S], BF16, tag="est")
                            nc.vector.tensor_copy(e_st[:], psx[:, :, 0:S])
                            nc.sync.dma_start(d_encT.ap()[b], e_st[:])
                    meanTb = ph.tile([128, 4, B], BF16, tag="mtb")
                    nc.scalar.mul(meanTb[:], ps_mean[:], 1.0 / S)

                # --- h0 = mean_w @ W_h0w.T + b_eff
                beff = ph.tile([1, H], BF16, tag="beff")
                nc.sync.dma_start(beff[:], p_beff[:])
                h_b16 = ph.tile([2 * B, 512], BF16, tag="hb16")
                for n in range(2):
                    w0 = ph.tile([128, 4, 512], BF16, tag="w0")
                    nc.sync.dma_start(
                        w0[:], p_wh0T.ap()
                        .rearrange("(kc p) h -> p kc h", p=128)[:, :, ts(n, 512)])
                    ps = php.tile([B, 512], F32, tag="hid")
                    for kc in range(4):
                        nc.tensor.matmul(ps[:], meanTb[:, kc], w0[:, kc],
                                         start=(kc == 0), stop=False)
                    nc.tensor.matmul(ps[:], ones1[0:1, :],
                                     beff[:, ts(n, 512)],
                                     start=False, stop=True)
                    nc.vector.tensor_copy(c_fold[n * B:(n + 1) * B, :], ps[:])
                    nc.vector.tensor_copy(h_b16[n * B:(n + 1) * B, :], ps[:])
                pst0 = php.tile([128, 8, B], BF16, tag="htp0")
                for kc in range(8):
                    hf, q = kc // 4, kc % 4
                    nc.tensor.transpose(
                        pst0[:, kc, :], h_b16[32 * hf:32 * (hf + 1), ts(q, 128)],
                        ident_b[32 * hf:32 * (hf + 1), 32 * hf:32 * (hf + 1)])
                nc.vector.tensor_copy(hsT[:, :, 0, :], pst0[:])

                # --- negmask from lengths (cheap, off critical path)
                lens = ph.tile([B, 1], F32, tag="lens")
                nc.sync.dma_start(lens[:], p_len[:])
                iota_i = ph.tile([B, S], I32, tag="iota")
                nc.gpsimd.iota(iota_i[:], pattern=[[1, S]], base=0,
                               channel_multiplier=0)
                iota_f = ph.tile([B, S], F32, tag="iotaf")
                nc.vector.tensor_copy(iota_f[:], iota_i[:])
                sg = ph.tile([B, S], F32, tag="sg")
                nc.vector.tensor_scalar(sg[:], iota_f[:], lens[:], None,
                                        ALU.subtract)
                halfc = ph.tile([B, 1], F32, tag="halfc")
                nc.gpsimd.memset(halfc, 0.5)
                negc = ph.tile([B, 1], F32, tag="negc")
                nc.gpsimd.memset(negc, NEG / 2)
                sg2 = ph.tile([B, S], F32, tag="sg2")
                nc.scalar.activation(sg2[:], sg[:], AF.Sign, bias=halfc[:])
                nc.scalar.activation(negm[:], sg2[:], AF.Identity,
                                     bias=negc[:], scale=NEG / 2)
                nc.sync.dma_start(d_negm[:], negm[:])

                # posT transposes (shared pos half of encT)
                for kc in range(4):
                    pst = php.tile([128, 128], BF16, tag="ptp")
                    nc.tensor.transpose(pst[:], pos_sb[:, ts(kc, 128)], ident_b)
                    nc.vector.tensor_copy(posT[:, kc], pst[:, 0:S])

            # ================= LSTM loop =================================
            with tc.tile_pool(name="wq1", bufs=1) as wq1, \
                 tc.tile_pool(name="p4s", bufs=2) as p4s, \
                 tc.tile_pool(name="pxb", bufs=2) as pxb, \
                 tc.tile_pool(name="p4ps", bufs=2, space="PSUM") as p4ps, \
                 tc.tile_pool(name="p4pt", bufs=1, space="PSUM") as p4pt, \
                 tc.tile_pool(name="p1ps", bufs=3, space="PSUM") as p1ps:

                # pass1 weights: kc 8..15, vocab cols 0..2047
                wp_q1 = wq1.tile([128, 8, 5, 512], BF16)
                nc.sync.dma_start(wp_q1[:], p_wp[:, 8:16, 0:5, :])

                XC = 8  # steps per transposed-gather chunk
                xbufs = {}

                def fetch_x(ch):
                    xb = pxb.tile([128, 4, XC * 32], BF16, tag="xb",
                                  name=f"xb{ch}")
                    nc.gpsimd.dma_gather(xb[:], p_emb_out[:, :],
                                         idx_tgt[:, ts(ch, TOK // 128)],
                                         TOK // 8, TOK // 8, E,
                                         transpose=True)
                    xbufs[ch] = xb

                def emit_x(t, pss):
                    """x + bias waves for step t (h-independent)."""
                    xb = xbufs[t // XC]
                    toff = (t % XC) * 32
                    for i in range(5):
                        for s_ in range(2):
                            ps = pss[s_]
                            for j in range(4):
                                n = N2COL[s_][j]
                                if i < 4:
                                    lhsT = xb[:, i, toff:toff + 32]
                                    rhs = wih[:, i, ts(n, 512)]
                                else:
                                    lhsT = ones1[:]
                                    rhs = bias_bc[0:1, ts(n, 512)]
                                nc.tensor.matmul(
                                    ps[32 * j:32 * (j + 1), :], lhsT, rhs,
                                    start=(i == 0), stop=False,
                                    skip_group_check=True,
                                    tile_position=(0, 32 * j))

                def emit_h(t, pss, acts):
                    """h waves: all kc of set0 first, then set1; then gate
                    activations per set."""
                    for s_ in range(2):
                        ps = pss[s_]
                        for kc in range(8):
                            for j in range(4):
                                n = N2COL[s_][j]
                                nc.tensor.matmul(
                                    ps[32 * j:32 * (j + 1), :], hsT[:, kc, t],
                                    whh[:, kc, ts(n, 512)],
                                    start=False, stop=(kc == 7),
                                    skip_group_check=True,
                                    tile_position=(0, 32 * j))
                        alo = p4s.tile([64, 512], BF16, tag=f"alo{s_}")
                        nc.scalar.activation(alo[:], ps[0:64, :], AF.Sigmoid)
                        ahi = p4s.tile([64, 512], BF16, tag=f"ahi{s_}")
                        nc.scalar.activation(ahi[:], ps[64:128, :],
                                             AF.Tanh if s_ == 0 else AF.Sigmoid)
                        acts.extend([alo, ahi])

                def emit_tail(t, acts):
                    """c/h update."""
                    gi_, gg, gf, go = acts
                    t2 = p4s.tile([64, 512], BF16, tag="t2")
                    nc.vector.tensor_mul(t2[:], gi_[:], gg[:])
                    nc.vector.tensor_mul(c_fold[:], gf[:], c_fold[:])
                    nc.vector.tensor_add(c_fold[:], c_fold[:], t2[:])
                    thc = p4s.tile([64, 512], BF16, tag="thc")
                    nc.scalar.activation(thc[:], c_fold[:], AF.Tanh)
                    h_fold = p4s.tile([64, 512], BF16, tag="hf")
                    nc.vector.tensor_mul(h_fold[:], go[:], thc[:])
                    return h_fold

                def emit_transposes(t, h_fold):
                    for g in range(2):
                        pst = p4pt.tile([128, 4, B], BF16, tag="htp")
                        for qq in range(4):
                            kc = g * 4 + qq
                            hf, q = kc // 4, kc % 4
                            nc.tensor.transpose(
                                pst[:, qq, :],
                                h_fold[32 * hf:32 * (hf + 1), ts(q, 128)],
                                ident_b[32 * hf:32 * (hf + 1),
                                        32 * hf:32 * (hf + 1)])
                        nc.vector.tensor_copy(
                            hsT[:, g * 4:(g + 1) * 4, t + 1, :], pst[:])

                def emit_pass1(m, vc):
                    """one pass1 chunk: spill[m, vc] = lstm_out @ Wp + bias."""
                    ps = p1ps.tile([128, 512], F32, tag="p1",
                                   name=f"p1_{m}_{vc}")
                    for kc in range(8):
                        nc.tensor.matmul(ps[:],
                                         hsT[:, kc, 1 + m * 4:5 + m * 4, :],
                                         wp_q1[:, kc, vc, :],
                                         start=(kc == 0), stop=(kc == 7))
                    sp = p4s.tile([128, 512], BF16, tag="sp")
                    nc.vector.scalar_tensor_tensor(
                        sp[:], ps[:], 1.0, bpw_lo[:, ts(vc, 512)],
                        ALU.mult, ALU.add)
                    nc.sync.dma_start(d_p1.ap()[m, vc], sp[:])

                NP1 = 80  # pass1 chunks: 16 m x 5 vc
                k_em = [0]

                def pace_pass1(t):
                    target = min(NP1 - 24, ((t - 3) * (NP1 - 24) + 59) // 60)
                    mmax = (t - 4) // 4
                    while k_em[0] < target and k_em[0] // 5 <= mmax:
                        emit_pass1(k_em[0] // 5, k_em[0] % 5)
                        k_em[0] += 1

                fetch_x(0)
                fetch_x(1)
                ps_cur = [p4ps.tile([128, 512], F32, tag=f"g{s_}",
                                    name=f"gates{s_}")
                          for s_ in range(2)]
                emit_x(0, ps_cur)
                for t in range(T if CUT >= 2 else 0):
                    acts = []
                    emit_h(t, ps_cur, acts)
                    if t + 1 < T:
                        if (t + 1) % XC == 0 and (t + 1) // XC + 1 < T // XC:
                            fetch_x((t + 1) // XC + 1)

                        ps_nxt = [p4ps.tile([128, 512], F32, tag=f"g{s_}",
                                            name=f"gates{s_}_{t + 1}")
                                  for s_ in range(2)]
                        emit_x(t + 1, ps_nxt)
                    # PE filler between emit_x and the tail-gated transposes
                    if t >= 4:
                        pace_pass1(t)
                    if t == T - 1:
                        # last step has no emit_x(t+1): fill its tail wait
                        # with leftover chunks that don't need hsT[T]
                        mmax = (t - 4) // 4
                        extra = 0
                        while (extra < 4 and k_em[0] < NP1
                               and k_em[0] // 5 <= mmax):
                            emit_pass1(k_em[0] // 5, k_em[0] % 5)
                            k_em[0] += 1
                            extra += 1
                    h_fold = emit_tail(t, acts)
                    emit_transposes(t, h_fold)
                    if t + 1 < T:
                        ps_cur = ps_nxt
                while k_em[0] < NP1:  # pass1 leftovers
                    emit_pass1(k_em[0] // 5, k_em[0] % 5)
                    k_em[0] += 1

        # ================= P5 (attention) + P6 pass2 =====================
        with tc.tile_pool(name="p56", bufs=1) as p56:
            combT = p56.tile([128, 8, T, B], BF16)
            wp_c1 = p56.tile([128, 8, 5, 512], BF16)   # kc 0..7, vc 0..4
            nc.sync.dma_start(wp_c1[:], p_wp[:, 0:8, 0:5, :])
            bpw_hi = p56.tile([128, 3 * 512], BF16)
            nc.sync.dma_start(bpw_hi[:], p_bpw[:, 2560:4096])

            with tc.tile_pool(name="p5c", bufs=1) as p5c, \
                 tc.tile_pool(name="p5", bufs=4) as p5, \
                 tc.tile_pool(name="p5e", bufs=3, space="PSUM") as p5e, \
                 tc.tile_pool(name="p5cx", bufs=2, space="PSUM") as p5cx:
                negm0 = p5c.tile([1, B * S], BF16)
                nc.sync.dma_start(negm0[:],
                                  d_negm.ap().rearrange("b s -> (b s)")[None, :])

                for b in range(B if CUT >= 3 else 0):
                    encw_b = p5.tile([128, E], BF16, tag="encwb")
                    nc.sync.dma_start(encw_b[:], d_encw[b])
                    encT_b = p5.tile([128, 4, S], BF16, tag="encTb")
                    nc.sync.dma_start(encT_b[:], d_encT.ap()[b])

                    ps_e = p5e.tile([T, S], F32, tag="eng")
                    for kc in range(8):
                        rhs = encT_b[:, kc] if kc < 4 else posT[:, kc - 4]
                        nc.tensor.matmul(ps_e[:], hsT[:, kc, 0:T, b], rhs,
                                         start=(kc == 0), stop=False)
                    nc.tensor.matmul(ps_e[:], ones_col[:],
                                     negm0[:, b * S:(b + 1) * S],
                                     start=False, stop=True)
                    expE = p5.tile([T, S], F32, tag="expE")
                    esum = p5.tile([T, 1], F32, tag="esum")
                    nc.scalar.activation(expE[:], ps_e[:], AF.Exp,
                                         accum_out=esum[:])
                    esc = p5.tile([T, 1], F32, tag="esc")
                    nc.scalar.mul(esc[:], esum[:], float(S))
                    erec = p5.tile([T, 1], F32, tag="erec")
                    nc.vector.reciprocal(erec[:], esc[:])
                    align = p5.tile([T, S], BF16, tag="align")
                    nc.scalar.activation(align[:], expE[:], AF.Copy,
                                         scale=erec[:])
                    ps_at = p5e.tile([128, T], BF16, tag="alT")
                    nc.tensor.transpose(ps_at[0:S, :], align[:],
                                        ident_b[0:T, 0:T])
                    alT = p5.tile([S, T], BF16, tag="alTs")
                    nc.vector.tensor_copy(alT[:], ps_at[0:S, :])
                    ps_c = p5cx.tile([128, 8, T], F32, tag="ctx")
                    for mc in range(8):
                        lhsT = (encw_b[0:S, ts(mc, 128)] if mc < 4
                                else pos_sb[0:S, ts(mc - 4, 128)])
                        nc.tensor.matmul(ps_c[:, mc, :], lhsT, alT[:],
                                         start=True, stop=True)
                    nc.vector.tensor_copy(combT[:, :, :, b], ps_c[:])

            # ---- pass2: phase A (vc 0..3, kc 0..7 + spill merge) --------
            with tc.tile_pool(name="p6w", bufs=1) as p6w, \
                 tc.tile_pool(name="p6", bufs=3) as p6, \
                 tc.tile_pool(name="p6s", bufs=2) as p6s, \
                 tc.tile_pool(name="p6ps", bufs=2, space="PSUM") as p6ps:
                # first spill block before the big phase-B weight DMA so it
                # isn't queued behind 8MB
                sp0 = p6s.tile([128, 4, 512], BF16, tag="spin", name="spin0")
                for vc in range(4):
                    nc.sync.dma_start(sp0[:, vc], d_p1.ap()[0, vc])
                # prefetch phase-B weights under phase A
                wp_c23 = p6w.tile([128, 16, 3, 512], BF16)
                nc.sync.dma_start(wp_c23[:], p_wp[:, :, 5:8, :])

                # kc-outer so the 4 vc matmuls share one stationary load
                for m in range(16 if CUT >= 4 else 0):
                    if m == 0:
                        sp_in = sp0
                    else:
                        sp_in = p6s.tile([128, 4, 512], BF16, tag="spin",
                                         name=f"spin{m}")
                        for vc in range(4):
                            nc.sync.dma_start(sp_in[:, vc], d_p1.ap()[m, vc])
                    pss = [p6ps.tile([128, 512], F32, tag=f"o{v}",
                                     name=f"psA_{m}_{v}") for v in range(4)]
                    for kc in range(8):
                        for vc in range(4):
                            nc.tensor.matmul(pss[vc][:],
                                             combT[:, kc, ts(m, 4), :],
                                             wp_c1[:, kc, vc, :],
                                             start=(kc == 0), stop=(kc == 7))
                    for vc in range(4):
                        o_sb = p6.tile([128, 512], F32, tag="osb")
                        nc.vector.scalar_tensor_tensor(
                            o_sb[:], pss[vc][:], 1.0, sp_in[:, vc],
                            ALU.mult, ALU.add)
                        nc.sync.dma_start(p_out[ts(m, 128), ts(vc, 512)],
                                          o_sb[:])

                # ---- pass2: phase B (vc4 spill-merge + vc 5..7 full) ----
                for m in range(16 if CUT >= 4 else 0):
                    sp4 = p6s.tile([128, 512], BF16, tag="sp4",
                                   name=f"sp4_{m}")
                    nc.sync.dma_start(sp4[:], d_p1.ap()[m, 4])
                    ps4 = p6ps.tile([128, 512], F32, tag="o0",
                                    name=f"psB4_{m}")
                    pss = [p6ps.tile([128, 512], F32, tag=f"o{v + 1}",
                                     name=f"psB_{m}_{v}") for v in range(3)]
                    for kc in range(16):
                        if kc < 8:
                            lhsT = combT[:, kc, ts(m, 4), :]
                        else:
                            lhsT = hsT[:, kc - 8, 1 + m * 4:1 + (m + 1) * 4, :]
                        if kc < 8:
                            nc.tensor.matmul(ps4[:], lhsT,
                                             wp_c1[:, kc, 4, :],
                                             start=(kc == 0), stop=(kc == 7))
                        for v in range(3):
                            nc.tensor.matmul(pss[v][:], lhsT,
                                             wp_c23[:, kc, v, :],
                                             start=(kc == 0), stop=(kc == 15))
                    o4 = p6.tile([128, 512], F32, tag="osb")
                    nc.vector.scalar_tensor_tensor(
                        o4[:], ps4[:], 1.0, sp4[:], ALU.mult, ALU.add)
                    nc.sync.dma_start(p_out[ts(m, 128), ts(4, 512)], o4[:])
                    for v in range(3):
                        o_sb = p6.tile([128, 512], F32, tag="osb")
                        nc.vector.scalar_tensor_tensor(
                            o_sb[:], pss[v][:], 1.0, bpw_hi[:, ts(v, 512)],
                            ALU.mult, ALU.add)
                        nc.sync.dma_start(p_out[ts(m, 128), ts(5 + v, 512)],
                                          o_sb[:])

    nc.finalize()
    return nc


_CACHED = {}
LAST_EXEC_NS = None


def prep_in_maps(inputs):
    import ml_dtypes
    bf16 = ml_dtypes.bfloat16

    src = np.asarray(inputs["source_sentences"]).astype(np.int64)
    lens = np.asarray(inputs["source_lengths"]).astype(np.float32).reshape(B, 1)
    tgt = np.asarray(inputs["target_sentences"]).astype(np.int64)
    emb_in = np.asarray(inputs["emb_in"], np.float32).astype(bf16)
    emb_out = np.asarray(inputs["emb_out"], np.float32).astype(bf16)
    pos_emb = np.asarray(inputs["pos_emb"], np.float32)[:S]
    w_h0 = np.asarray(inputs["W_h0"], np.float32)         # [H, 2E]
    b_h0 = np.asarray(inputs["b_h0"], np.float32)
    # fold the (batch-independent) positional mean through W_h0 into the bias
    pos_mean = pos_emb.mean(axis=0)                       # [E]
    b_eff = b_h0 + w_h0[:, E:] @ pos_mean                 # [H]
    wihT = np.asarray(inputs["W_ih"], np.float32).T       # [E, 4H]
    whhT = np.asarray(inputs["W_hh"], np.float32).T       # [H, 4H]
    bsum = (np.asarray(inputs["b_ih"], np.float32)
            + np.asarray(inputs["b_hh"], np.float32)).reshape(1, G4)
    wproj = np.asarray(inputs["W_proj"], np.float32)
    bproj = np.asarray(inputs["b_proj"], np.float32)

    src_pad = np.zeros((B, SP), np.int64)
    src_pad[:, :S] = src
    idx_src = _wrap_idx(src_pad.reshape(-1))
    idx_tgt = _wrap_idx(tgt.T.reshape(-1))                # t-major: j = t*32+b

    common = dict(
        idx_src=idx_src, idx_tgt=idx_tgt,
        emb_in=np.ascontiguousarray(emb_in),
        emb_out=np.ascontiguousarray(emb_out),
        pos=np.ascontiguousarray(pos_emb.astype(bf16)),
        wh0T=np.ascontiguousarray(w_h0[:, :E].T.astype(bf16)),
        beff=np.ascontiguousarray(b_eff.reshape(1, H).astype(bf16)),
        wihT=np.ascontiguousarray(
            wihT.reshape(4, 128, G4).transpose(1, 0, 2).astype(bf16)),
        whhT=np.ascontiguousarray(
            whhT.reshape(8, 128, G4).transpose(1, 0, 2).astype(bf16)),
        bsum=np.ascontiguousarray(bsum.astype(bf16)),
        lens=lens,
    )
    in_maps = []
    for c in range(NCORES):
        wp = wproj[c * VSH:(c + 1) * VSH]                 # [4000, 2048]
        wpT = np.zeros((2 * E + H, VSHP), np.float32)
        wpT[:, :VSH] = wp.T
        # pack [kc*128+p, vc8*512+v] -> [p, kc, vc8, v]
        wp_pk = np.ascontiguousarray(
            wpT.reshape(16, 128, 8, 512).transpose(1, 0, 2, 3).astype(bf16))
        bp = np.zeros((VSHP,), np.float32)
        bp[:VSH] = bproj[c * VSH:(c + 1) * VSH]
        bpw = np.ascontiguousarray(
            np.tile(bp.astype(bf16)[None, :], (128, 1)))
        in_maps.append(dict(common, wp=wp_pk, bpw=bpw))

    return in_maps


def kernel(**inputs) -> np.ndarray:
    in_maps = prep_in_maps(inputs)
    if "nc" not in _CACHED:
        _CACHED["nc"] = build_nc()
    nc = _CACHED["nc"]
    trace = bool(int(os.environ.get("KTRACE", "0")))
    tmpdir = os.environ.get("KTRACE_DIR") or None
    res = run_bass_kernel_spmd(nc, in_maps, list(range(NCORES)),
                               trace=trace, tmpdir=tmpdir)
    global LAST_EXEC_NS
    LAST_EXEC_NS = res.exec_time_ns
    outs = []
    for c in range(NCORES):
        o = res.results[c]["out"]                  # [TOK, VSHP] tok=t*32+b
        o = o.reshape(T, B, VSHP)[:, :, :VSH].transpose(1, 0, 2)
        outs.append(o)                             # [B, T, 4000]
    return np.concatenate(outs, axis=2)


if __name__ == "__main__":
    build_nc()
    print("build ok")
